# revision 3
# baseline (speedup 1.0000x reference)
"""Trainium2 Bass kernel for nn_ClusteringLayer (vq_codebook, Student-t cluster assignment).

Computes, for x [65536, 512] and centroids [512, 512]:
    d2 = ||x||^2 + ||c||^2 - 2 x @ c^T          # [N, K] squared distances
    q  = 1 / (1 + d2); q = q / q.sum(axis=1)    # row-normalized Student-t kernel

Sharding: data-parallel over the N axis across 8 NeuronCores (8192 rows each),
centroids replicated. No collectives needed.

fp8 DoubleRow formulation (mm="dr", default):
  Host ships xw = (512*w) * x quantized e4m3 (w = 1/(1+||x||^2) folds into the
  fp8 rounding for free), and cq = -2 C^T e4m3. Per [128, 4x512] block:
    PE : per 128-row tile, 2 fp8 DoubleRow matmuls (256-deep contraction each)
         + K=4 bf16 aug matmul adding 512*w*c_sq (coarse+residual on both sides)
         -> psum = 512*w*(c_sq - 2 x.c)
    ACT: ONE grouped Reciprocal over all 4 psum banks:
         qu = 1/(psum/512 + 1) = A/(1+d2), scale+bias immediates -> fp16
    DVE: per tile: copy+accum (tensor_scalar, 4x mode) -> s = sum(qu)
         one batched reciprocal_approx_fast over s4 [128,4]
         per tile: qu * rs -> fp16 out (tensor_scalar, 4x mode)
The per-row factor A = 1+||x||^2 cancels in the normalization.
"""

import numpy as np
from contextlib import ExitStack

try:
    from concourse import bacc, bass, tile, mybir
except ImportError:  # container layout: concourse lives in /opt/trn_rl_repo
    import sys

    sys.path.insert(0, "/opt/trn_rl_repo")
    from concourse import bacc, bass, tile, mybir

from concourse.bass_utils import run_bass_kernel_spmd
import ml_dtypes

P = 128
D = 512  # feature dim
KC = 512  # number of centroids
NCORES = 8
N_FULL = 65536
N_SHARD = N_FULL // NCORES  # 8192
BLK = 512  # x rows per block (4 output tiles, 4 psum banks)
NCH = D // P  # 4 contraction chunks
G = BLK // P  # tiles per block / psum group

F32 = mybir.dt.float32
BF16 = mybir.dt.bfloat16
F16 = mybir.dt.float16
FP8 = mybir.dt.float8e4

E4M3 = ml_dtypes.float8_e4m3


def _act_unsafe(nc, out, in_, func, bias=0.0, scale=1.0, accum_out=None):
    """activation() without the Reciprocal accuracy guard (validated on HW for
    this kernel's argument range ~[1.0, 3.6])."""
    se = nc.scalar
    ins_ = [se.lower_ap(in_)]
    for arg in (bias, scale, 0.0):
        if isinstance(arg, bass.AP):
            ins_.append(se.lower_ap(arg))
        else:
            ins_.append(mybir.ImmediateValue(dtype=mybir.dt.float32, value=float(arg)))
    outs_ = [se.lower_ap(out)]
    if accum_out is not None:
        outs_.append(se.lower_ap(accum_out))
    return se.add_instruction(
        mybir.InstActivation(
            name=nc.get_next_instruction_name(), func=func, ins=ins_, outs=outs_
        )
    )


def build_nc(n_rows=N_SHARD, repeat=1, enable_asserts=False):
    """Build + compile the SPMD Bass module for one core's shard of n_rows."""
    assert n_rows % BLK == 0
    nblk = n_rows // BLK
    ntile = n_rows // P

    nc = bacc.Bacc(
        "TRN2",
        target_bir_lowering=False,
        debug=False,
        enable_asserts=enable_asserts,
        num_devices=NCORES,
    )
    xw = nc.dram_tensor("xw", [D, n_rows], FP8, kind="ExternalInput").ap()
    cq = nc.dram_tensor("cq", [D, KC], FP8, kind="ExternalInput").ap()
    waug = nc.dram_tensor("waug", [4, n_rows], BF16, kind="ExternalInput").ap()
    caug = nc.dram_tensor("caug", [4, KC], BF16, kind="ExternalInput").ap()
    q = nc.dram_tensor("q", [n_rows, KC], F16, kind="ExternalOutput").ap()

    MUL = mybir.AluOpType.mult
    DR = mybir.MatmulPerfMode.DoubleRow

    with tile.TileContext(nc) as tc, ExitStack() as ctx:
        const = ctx.enter_context(tc.tile_pool(name="const", bufs=1))
        psum_pool = ctx.enter_context(tc.tile_pool(name="psum", bufs=2, space="PSUM"))
        xt_pool = ctx.enter_context(tc.tile_pool(name="xtp", bufs=3))
        qu_pool = ctx.enter_context(tc.tile_pool(name="qup", bufs=3))
        z_pool = ctx.enter_context(tc.tile_pool(name="zp", bufs=2))
        stat_pool = ctx.enter_context(tc.tile_pool(name="stat", bufs=8))
        out_pool = ctx.enter_context(tc.tile_pool(name="outp", bufs=3))

        # ---------------- prologue: load replicated constants ----------------
        ctb = const.tile([P, NCH, KC], FP8)  # -2 C^T fp8, chunked on partitions
        nc.sync.dma_start(ctb[:], cq.rearrange("(c p) k -> p c k", p=P))
        waug_t = const.tile([4, ntile, P], BF16)  # [wc,wc,wr,wr] rows per tile
        nc.sync.dma_start(waug_t[:], waug.rearrange("k (t p) -> k t p", p=P))
        caug_t = const.tile([4, KC], BF16)  # [csq_c, csq_r, csq_c, csq_r]
        nc.sync.dma_start(caug_t[:], caug[:])

        # ---------------- main loop ----------------
        for _ in range(repeat):
            for b in range(nblk):
                xtb = xt_pool.tile([P, NCH, BLK], FP8)
                nc.sync.dma_start(
                    xtb[:],
                    xw[:, b * BLK : (b + 1) * BLK].rearrange("(c p) m -> p c m", p=P),
                )
                ps = psum_pool.tile([P, G, KC], F32)
                for j in range(G):
                    t = b * G + j
                    # psum = 512*w*(-2 x.c) over D=512 in 2 fp8 DoubleRow matmuls
                    nc.tensor.matmul(
                        ps[:, j, :],
                        xtb[:, 0:2, j * P : (j + 1) * P],
                        ctb[:, 0:2, :],
                        start=True,
                        stop=False,
                        perf_mode=DR,
                    )
                    nc.tensor.matmul(
                        ps[:, j, :],
                        xtb[:, 2:4, j * P : (j + 1) * P],
                        ctb[:, 2:4, :],
                        start=False,
                        stop=False,
                        perf_mode=DR,
                    )
                    # += 512*w*c_sq via K=4 outer product (coarse+resid both sides)
                    nc.tensor.matmul(
                        ps[:, j, :],
                        waug_t[:, t, :],
                        caug_t[:],
                        start=False,
                        stop=True,
                    )
                # qu = 1/(ps/512 + 1) = A/(1+d2), one ACT over all 4 banks
                qu = qu_pool.tile([P, G, KC], F16)
                _act_unsafe(
                    nc,
                    qu[:],
                    ps[:],
                    mybir.ActivationFunctionType.Reciprocal,
                    bias=1.0,
                    scale=1.0 / 512.0,
                )
                # row sums s[j] = sum_k qu (tensor_scalar copy + accum, 4x mode)
                s4 = stat_pool.tile([P, G], F32)
                z = z_pool.tile([P, G, KC], F16)
                for j in range(G):
                    nc.vector.tensor_scalar(
                        z[:, j, :],
                        qu[:, j, :],
                        1.0,
                        0.0,
                        MUL,
                        mybir.AluOpType.add,
                        accum_out=s4[:, j : j + 1],
                    )
                rs4 = stat_pool.tile([P, G], F32)
                nc.vector.reciprocal_approx_fast(rs4[:], s4[:])
                ob = out_pool.tile([P, G, KC], F16)
                for j in range(G):
                    nc.vector.tensor_scalar_mul(
                        ob[:, j, :], qu[:, j, :], rs4[:, j : j + 1]
                    )
                nc.sync.dma_start(
                    q[b * BLK : (b + 1) * BLK, :].rearrange("(j p) k -> p j k", p=P),
                    ob[:],
                )

    nc.compile()
    return nc


_NC_CACHE = {}


def _get_nc(**kw):
    key = tuple(sorted(kw.items()))
    if key not in _NC_CACHE:
        _NC_CACHE[key] = build_nc(**kw)
    return _NC_CACHE[key]


def prep_inputs(x, centroids):
    """Host-side layout prep + per-core sharding."""
    xf = np.ascontiguousarray(np.asarray(x, dtype=np.float32))
    cf = np.asarray(centroids, dtype=np.float32)
    n = xf.shape[0]
    ns = n // NCORES

    w512 = 512.0 / (1.0 + (xf * xf).sum(1))  # [N] f32
    xwT = np.ascontiguousarray((xf * w512[:, None]).T.astype(E4M3))  # [D, N] fp8
    cqT = np.ascontiguousarray((-2.0 * cf.T).astype(E4M3))  # [D, K] fp8

    wc = w512.astype(ml_dtypes.bfloat16)
    wr = (w512 - wc.astype(np.float32)).astype(ml_dtypes.bfloat16)
    waug = np.ascontiguousarray(np.stack([wc, wc, wr, wr], axis=0))  # [4, N] bf16

    csq = (cf * cf).sum(1)  # [K] f32 exact
    cc = csq.astype(ml_dtypes.bfloat16)
    cr = (csq - cc.astype(np.float32)).astype(ml_dtypes.bfloat16)
    caug = np.ascontiguousarray(np.stack([cc, cr, cc, cr], axis=0))  # [4, K] bf16

    in_maps = []
    for c in range(NCORES):
        in_maps.append(
            {
                "xw": np.ascontiguousarray(xwT[:, c * ns : (c + 1) * ns]),
                "cq": cqT,
                "waug": np.ascontiguousarray(waug[:, c * ns : (c + 1) * ns]),
                "caug": caug,
            }
        )
    return in_maps


def kernel(x, centroids):
    nc = _get_nc()
    in_maps = prep_inputs(x, centroids)
    res = run_bass_kernel_spmd(nc, in_maps, core_ids=list(range(NCORES)))
    out = np.concatenate([res.results[c]["q"] for c in range(NCORES)], axis=0)
    return out.astype(np.float32)


if __name__ == "__main__":
    # smoke test with random data (no reference available standalone)
    rng = np.random.default_rng(0)
    x = rng.standard_normal((N_FULL, D), dtype=np.float32)
    c = rng.standard_normal((KC, D), dtype=np.float32)
    q = kernel(x, c)
    print("q", q.shape, q.dtype, q.sum(axis=1)[:4])
    xs = (x * x).sum(1)[:, None]
    cs = (c * c).sum(1)[None, :]
    d2 = xs + cs - 2.0 * (x @ c.T)
    qe = 1.0 / (1.0 + d2)
    qe /= qe.sum(1, keepdims=True)
    re = np.abs(q - qe) / np.maximum(np.abs(qe), 1e-30)
    print("rel err vs local reference:", re.max())


# revision 10
# speedup vs baseline: 1.2751x; 1.2751x over previous
"""Trainium2 Bass kernel for nn_ClusteringLayer (vq_codebook, Student-t cluster assignment).

Computes, for x [65536, 512] and centroids [512, 512]:
    d2 = ||x||^2 + ||c||^2 - 2 x @ c^T          # [N, K] squared distances
    q  = 1 / (1 + d2); q = q / q.sum(axis=1)    # row-normalized Student-t kernel

Sharding: data-parallel over the N axis across 8 NeuronCores (8192 rows each),
centroids replicated. No collectives needed.

Formulation (v3):
  Host ships xw = (512*w) * x quantized e4m3 (w = 1/(1+||x||^2) folds into the
  fp8 rounding for free) in SBUF-blocked layout, and cq = -2 C^T e4m3.
  Per [128, 512] output tile:
    PE : 4 fp8 matmuls (contraction 128 each)
         + K=4 bf16 aug matmul adding 512*w*c_sq (coarse+residual both sides)
         -> psum = 512*w*(c_sq - 2 x.c)   [one bank per tile, 8 banks cycling]
    ACT: qu = Recip(psum/512 + 1) = A/(1+d2) -> fp16, scale+bias immediates
    DVE: copy+accum (tensor_scalar, 4x mode) -> s = sum_k(qu)
         rs = 1/s (reciprocal_approx_fast), out = qu*rs -> fp16 (4x mode)
  The per-row factor A = 1+||x||^2 cancels in the normalization.
  DMA: input and output use host-side blocked layouts so every partition line
  is 2KB/4KB contiguous; output is de-blocked on the host after gather.
"""

import numpy as np
from contextlib import ExitStack

try:
    from concourse import bacc, bass, tile, mybir
except ImportError:  # container layout: concourse lives in /opt/trn_rl_repo
    import sys

    sys.path.insert(0, "/opt/trn_rl_repo")
    from concourse import bacc, bass, tile, mybir

from concourse.bass_utils import run_bass_kernel_spmd
import ml_dtypes

P = 128
D = 512  # feature dim
KC = 512  # number of centroids
NCORES = 8
N_FULL = 65536
N_SHARD = N_FULL // NCORES  # 8192
BLK = 512  # x rows per DMA block (4 output tiles)
NCH = D // P  # 4 contraction chunks
G = BLK // P  # tiles per DMA block

F32 = mybir.dt.float32
BF16 = mybir.dt.bfloat16
F16 = mybir.dt.float16
FP8 = mybir.dt.float8e4

E4M3 = ml_dtypes.float8_e4m3


def _act_unsafe(nc, out, in_, func, bias=0.0, scale=1.0, accum_out=None):
    """activation() without the Reciprocal accuracy guard (validated on HW for
    this kernel's argument range ~[1.0, 3.6])."""
    se = nc.scalar
    ins_ = [se.lower_ap(in_)]
    for arg in (bias, scale, 0.0):
        if isinstance(arg, bass.AP):
            ins_.append(se.lower_ap(arg))
        else:
            ins_.append(mybir.ImmediateValue(dtype=mybir.dt.float32, value=float(arg)))
    outs_ = [se.lower_ap(out)]
    if accum_out is not None:
        outs_.append(se.lower_ap(accum_out))
    return se.add_instruction(
        mybir.InstActivation(
            name=nc.get_next_instruction_name(), func=func, ins=ins_, outs=outs_
        )
    )


def build_nc(n_rows=N_SHARD, repeat=1, enable_asserts=False, mm="fp8", ablate=()):
    """Build + compile the SPMD Bass module for one core's shard of n_rows.

    mm: "fp8" = 4 fp8 matmuls (default), "bf16" = 4 bf16 matmuls.
    ablate: perf-experiment switches (break numerics, timing only):
      'epi'   — skip ACT+DVE epilogue, DMA out gpsimd-memset ob
      'mm'    — skip main matmuls (keep aug as psum writer)
      'indma' — skip per-block x DMA; matmuls read a const tile (pure-PE probe)
    """
    ablate = set(ablate)
    MMD = {"fp8": FP8, "bf16": BF16}[mm]
    assert n_rows % BLK == 0
    nblk = n_rows // BLK
    ntile = n_rows // P

    nc = bacc.Bacc(
        "TRN2",
        target_bir_lowering=False,
        debug=False,
        enable_asserts=enable_asserts,
        num_devices=NCORES,
    )
    # xw blocked: [nblk, P, NCH*BLK]; partition line = NCH*BLK bytes contiguous
    xw = nc.dram_tensor("xw", [nblk, P, NCH * BLK], MMD, kind="ExternalInput").ap()
    cq = nc.dram_tensor("cq", [D, KC], MMD, kind="ExternalInput").ap()
    waug = nc.dram_tensor("waug", [4, n_rows], BF16, kind="ExternalInput").ap()
    caug = nc.dram_tensor("caug", [4, KC], BF16, kind="ExternalInput").ap()
    # q blocked: [nblk, P, G*KC] fp16; host de-blocks after gather
    q = nc.dram_tensor("q", [nblk, P, G * KC], F16, kind="ExternalOutput").ap()

    MUL = mybir.AluOpType.mult

    with tile.TileContext(nc) as tc, ExitStack() as ctx:
        const = ctx.enter_context(tc.tile_pool(name="const", bufs=1))
        psum_pool = ctx.enter_context(tc.tile_pool(name="psum", bufs=8, space="PSUM"))
        xt_pool = ctx.enter_context(tc.tile_pool(name="xtp", bufs=3))
        qu_pool = ctx.enter_context(tc.tile_pool(name="qup", bufs=8))
        z_pool = ctx.enter_context(tc.tile_pool(name="zp", bufs=4))
        stat_pool = ctx.enter_context(tc.tile_pool(name="stat", bufs=8))
        out_pool = ctx.enter_context(tc.tile_pool(name="outp", bufs=3))

        # ---------------- prologue: load replicated constants ----------------
        ctb = const.tile([P, NCH, KC], MMD)  # -2 C^T, chunked on partitions
        nc.sync.dma_start(ctb[:], cq.rearrange("(c p) k -> p c k", p=P))
        waug_t = const.tile([4, ntile, P], BF16)  # [wc,wc,wr,wr] rows per tile
        nc.sync.dma_start(waug_t[:], waug.rearrange("k (t p) -> k t p", p=P))
        caug_t = const.tile([4, KC], BF16)  # [csq_c, csq_r, csq_c, csq_r]
        nc.sync.dma_start(caug_t[:], caug[:])
        if "indma" in ablate:
            xconst = const.tile([P, NCH, BLK], MMD)
            nc.gpsimd.memset(xconst[:], 0.25)

        # ---------------- main loop ----------------
        for _ in range(repeat):
            for b in range(nblk):
                if "indma" not in ablate:
                    xtb = xt_pool.tile([P, NCH, BLK], MMD)
                    nc.sync.dma_start(
                        xtb[:],
                        xw[b].rearrange("p (c m) -> p c m", c=NCH),
                    )
                else:
                    xtb = xconst
                ob = out_pool.tile([P, G, KC], F16)
                if "epi" in ablate:
                    nc.gpsimd.memset(ob[:], 0.5)
                for j in range(G):
                    t = b * G + j
                    ps = psum_pool.tile([P, KC], F32)
                    if "mm" not in ablate:
                        # psum = 512*w*(-2 x.c) over D=512
                        for c in range(NCH):
                            nc.tensor.matmul(
                                ps[:],
                                xtb[:, c, j * P : (j + 1) * P],
                                ctb[:, c, :],
                                start=(c == 0),
                                stop=False,
                            )
                        # += 512*w*c_sq via K=4 outer product (coarse+resid)
                        nc.tensor.matmul(
                            ps[:], waug_t[:, t, :], caug_t[:], start=False, stop=True
                        )
                    else:
                        nc.tensor.matmul(
                            ps[:], waug_t[:, t, :], caug_t[:], start=True, stop=True
                        )
                    if "epi" in ablate:
                        continue
                    # qu = 1/(ps/512 + 1) = A/(1+d2); immediates only
                    qu = qu_pool.tile([P, KC], F16)
                    _act_unsafe(
                        nc,
                        qu[:],
                        ps[:],
                        mybir.ActivationFunctionType.Reciprocal,
                        bias=1.0,
                        scale=1.0 / 512.0,
                    )
                    # s = sum_k qu (tensor_scalar copy+accum, 4x mode)
                    s = stat_pool.tile([P, 1], F32)
                    z = z_pool.tile([P, KC], F16)
                    nc.vector.tensor_scalar(
                        z[:],
                        qu[:],
                        1.0,
                        0.0,
                        MUL,
                        mybir.AluOpType.add,
                        accum_out=s[:],
                    )
                    rs = stat_pool.tile([P, 1], F32)
                    nc.vector.reciprocal_approx_fast(rs[:], s[:])
                    nc.vector.tensor_scalar_mul(ob[:, j, :], qu[:], rs[:])
                nc.sync.dma_start(q[b], ob[:].rearrange("p j k -> p (j k)"))

    nc.compile()
    return nc


_NC_CACHE = {}


def _get_nc(**kw):
    key = tuple(sorted(kw.items()))
    if key not in _NC_CACHE:
        _NC_CACHE[key] = build_nc(**kw)
    return _NC_CACHE[key]


def prep_inputs(x, centroids, mm="fp8"):
    """Host-side layout prep + per-core sharding."""
    MMD = {"fp8": E4M3, "bf16": ml_dtypes.bfloat16}[mm]
    xf = np.ascontiguousarray(np.asarray(x, dtype=np.float32))
    cf = np.asarray(centroids, dtype=np.float32)
    n = xf.shape[0]
    ns = n // NCORES
    nblk = ns // BLK

    w512 = 512.0 / (1.0 + (xf * xf).sum(1))  # [N] f32
    xwv = (xf * w512[:, None]).astype(MMD)  # [N, D]
    cqT = np.ascontiguousarray((-2.0 * cf.T).astype(MMD))  # [D, K]

    wc = w512.astype(ml_dtypes.bfloat16)
    wr = (w512 - wc.astype(np.float32)).astype(ml_dtypes.bfloat16)
    waug = np.ascontiguousarray(np.stack([wc, wc, wr, wr], axis=0))  # [4, N] bf16

    csq = (cf * cf).sum(1)  # [K] f32 exact
    cc = csq.astype(ml_dtypes.bfloat16)
    cr = (csq - cc.astype(np.float32)).astype(ml_dtypes.bfloat16)
    caug = np.ascontiguousarray(np.stack([cc, cr, cc, cr], axis=0))  # [4, K] bf16

    in_maps = []
    for c in range(NCORES):
        # blocked xw: xw_b[b, p, c*BLK+m] = xw[row b*BLK+m, d=c*128+p]
        xs = xwv[c * ns : (c + 1) * ns]  # [ns, D]
        xb = np.ascontiguousarray(
            xs.reshape(nblk, BLK, NCH, P).transpose(0, 3, 2, 1).reshape(nblk, P, NCH * BLK)
        )
        in_maps.append(
            {
                "xw": xb,
                "cq": cqT,
                "waug": np.ascontiguousarray(waug[:, c * ns : (c + 1) * ns]),
                "caug": caug,
            }
        )
    return in_maps


def unblock_q(qb, n_rows=N_SHARD):
    """[nblk, P, G*KC] fp16 -> [n_rows, KC]"""
    nblk = n_rows // BLK
    return qb.reshape(nblk, P, G, KC).transpose(0, 2, 1, 3).reshape(n_rows, KC)


def kernel(x, centroids):
    nc = _get_nc()
    in_maps = prep_inputs(x, centroids)
    res = run_bass_kernel_spmd(nc, in_maps, core_ids=list(range(NCORES)))
    out = np.concatenate(
        [unblock_q(res.results[c]["q"]) for c in range(NCORES)], axis=0
    )
    return out.astype(np.float32)


if __name__ == "__main__":
    # smoke test with random data (no reference available standalone)
    rng = np.random.default_rng(0)
    x = rng.standard_normal((N_FULL, D), dtype=np.float32)
    c = rng.standard_normal((KC, D), dtype=np.float32)
    q = kernel(x, c)
    print("q", q.shape, q.dtype, q.sum(axis=1)[:4])
    xs = (x * x).sum(1)[:, None]
    cs = (c * c).sum(1)[None, :]
    d2 = xs + cs - 2.0 * (x @ c.T)
    qe = 1.0 / (1.0 + d2)
    qe /= qe.sum(1, keepdims=True)
    re = np.abs(q - qe) / np.maximum(np.abs(qe), 1e-30)
    print("rel err vs local reference:", re.max())


# revision 12
# speedup vs baseline: 1.2763x; 1.0010x over previous
"""Trainium2 Bass kernel for nn_ClusteringLayer (vq_codebook, Student-t cluster assignment).

Computes, for x [65536, 512] and centroids [512, 512]:
    d2 = ||x||^2 + ||c||^2 - 2 x @ c^T          # [N, K] squared distances
    q  = 1 / (1 + d2); q = q / q.sum(axis=1)    # row-normalized Student-t kernel

Sharding: data-parallel over the N axis across 8 NeuronCores (8192 rows each),
centroids replicated. No collectives needed.

Formulation (v3):
  Host ships xw = (512*w) * x quantized e4m3 (w = 1/(1+||x||^2) folds into the
  fp8 rounding for free) in SBUF-blocked layout, and cq = -2 C^T e4m3.
  Per [128, 512] output tile:
    PE : 4 fp8 matmuls (contraction 128 each)
         + K=4 bf16 aug matmul adding 512*w*c_sq (coarse+residual both sides)
         -> psum = 512*w*(c_sq - 2 x.c)   [one bank per tile, 8 banks cycling]
    ACT: qu = Recip(psum/512 + 1) = A/(1+d2) -> fp16, scale+bias immediates
    DVE: copy+accum (tensor_scalar, 4x mode) -> s = sum_k(qu)
         rs = 1/s (reciprocal_approx_fast), out = qu*rs -> fp16 (4x mode)
  The per-row factor A = 1+||x||^2 cancels in the normalization.
  DMA: input and output use host-side blocked layouts so every partition line
  is 2KB/4KB contiguous; output is de-blocked on the host after gather.
"""

import numpy as np
from contextlib import ExitStack

try:
    from concourse import bacc, bass, tile, mybir
except ImportError:  # container layout: concourse lives in /opt/trn_rl_repo
    import sys

    sys.path.insert(0, "/opt/trn_rl_repo")
    from concourse import bacc, bass, tile, mybir

from concourse.bass_utils import run_bass_kernel_spmd
import ml_dtypes

P = 128
D = 512  # feature dim
KC = 512  # number of centroids
NCORES = 8
N_FULL = 65536
N_SHARD = N_FULL // NCORES  # 8192
BLK = 512  # x rows per DMA block (4 output tiles)
NCH = D // P  # 4 contraction chunks
G = BLK // P  # tiles per DMA block

F32 = mybir.dt.float32
BF16 = mybir.dt.bfloat16
F16 = mybir.dt.float16
FP8 = mybir.dt.float8e4

E4M3 = ml_dtypes.float8_e4m3


def _act_unsafe(nc, out, in_, func, bias=0.0, scale=1.0, accum_out=None):
    """activation() without the Reciprocal accuracy guard (validated on HW for
    this kernel's argument range ~[1.0, 3.6])."""
    se = nc.scalar
    ins_ = [se.lower_ap(in_)]
    for arg in (bias, scale, 0.0):
        if isinstance(arg, bass.AP):
            ins_.append(se.lower_ap(arg))
        else:
            ins_.append(mybir.ImmediateValue(dtype=mybir.dt.float32, value=float(arg)))
    outs_ = [se.lower_ap(out)]
    if accum_out is not None:
        outs_.append(se.lower_ap(accum_out))
    return se.add_instruction(
        mybir.InstActivation(
            name=nc.get_next_instruction_name(), func=func, ins=ins_, outs=outs_
        )
    )


def build_nc(n_rows=N_SHARD, repeat=1, enable_asserts=False, mm="fp8", ablate=()):
    """Build + compile the SPMD Bass module for one core's shard of n_rows.

    mm: "fp8" = 4 fp8 matmuls (default), "bf16" = 4 bf16 matmuls.
    ablate: perf-experiment switches (break numerics, timing only):
      'epi'   — skip ACT+DVE epilogue, DMA out gpsimd-memset ob
      'mm'    — skip main matmuls (keep aug as psum writer)
      'indma' — skip per-block x DMA; matmuls read a const tile (pure-PE probe)
    """
    ablate = set(ablate)
    MMD = {"fp8": FP8, "bf16": BF16}[mm]
    assert n_rows % BLK == 0
    nblk = n_rows // BLK
    ntile = n_rows // P

    nc = bacc.Bacc(
        "TRN2",
        target_bir_lowering=False,
        debug=False,
        enable_asserts=enable_asserts,
        num_devices=NCORES,
    )
    # xw blocked: [nblk, P, NCH*BLK]; partition line = NCH*BLK bytes contiguous
    xw = nc.dram_tensor("xw", [nblk, P, NCH * BLK], MMD, kind="ExternalInput").ap()
    cq = nc.dram_tensor("cq", [D, KC], MMD, kind="ExternalInput").ap()
    waug = nc.dram_tensor("waug", [4, n_rows], BF16, kind="ExternalInput").ap()
    caug = nc.dram_tensor("caug", [4, KC], BF16, kind="ExternalInput").ap()
    # q blocked: [nblk, P, G*KC] fp16; host de-blocks after gather
    q = nc.dram_tensor("q", [nblk, P, G * KC], F16, kind="ExternalOutput").ap()

    MUL = mybir.AluOpType.mult

    with tile.TileContext(nc) as tc, ExitStack() as ctx:
        const = ctx.enter_context(tc.tile_pool(name="const", bufs=1))
        psum_pool = ctx.enter_context(tc.tile_pool(name="psum", bufs=4, space="PSUM"))
        xt_pool = ctx.enter_context(tc.tile_pool(name="xtp", bufs=4))
        qu_pool = ctx.enter_context(tc.tile_pool(name="qup", bufs=4))
        z_pool = ctx.enter_context(tc.tile_pool(name="zp", bufs=4))
        stat_pool = ctx.enter_context(tc.tile_pool(name="stat", bufs=8))
        out_pool = ctx.enter_context(tc.tile_pool(name="outp", bufs=3))

        # ---------------- prologue: load replicated constants ----------------
        ctb = const.tile([P, NCH, KC], MMD)  # -2 C^T, chunked on partitions
        nc.sync.dma_start(ctb[:], cq.rearrange("(c p) k -> p c k", p=P))
        waug_t = const.tile([4, ntile, P], BF16)  # [wc,wc,wr,wr] rows per tile
        nc.sync.dma_start(waug_t[:], waug.rearrange("k (t p) -> k t p", p=P))
        caug_t = const.tile([4, KC], BF16)  # [csq_c, csq_r, csq_c, csq_r]
        nc.sync.dma_start(caug_t[:], caug[:])
        if "indma" in ablate:
            xconst = const.tile([P, NCH, BLK], MMD)
            nc.gpsimd.memset(xconst[:], 0.25)

        # ---------------- main loop ----------------
        for _ in range(repeat):
            for b in range(nblk):
                if "indma" not in ablate:
                    xtb = xt_pool.tile([P, NCH, BLK], MMD)
                    nc.sync.dma_start(
                        xtb[:],
                        xw[b].rearrange("p (c m) -> p c m", c=NCH),
                    )
                else:
                    xtb = xconst
                ob = out_pool.tile([P, G, KC], F16)
                if "epi" in ablate:
                    nc.gpsimd.memset(ob[:], 0.5)
                for jp in range(G // 2):
                    ps = psum_pool.tile([P, 2, KC], F32)
                    for jj in range(2):
                        j = 2 * jp + jj
                        t = b * G + j
                        if "mm" not in ablate:
                            # psum = 512*w*(-2 x.c) over D=512
                            for c in range(NCH):
                                nc.tensor.matmul(
                                    ps[:, jj, :],
                                    xtb[:, c, j * P : (j + 1) * P],
                                    ctb[:, c, :],
                                    start=(c == 0),
                                    stop=False,
                                )
                            # += 512*w*c_sq via K=4 outer product (coarse+resid)
                            nc.tensor.matmul(
                                ps[:, jj, :],
                                waug_t[:, t, :],
                                caug_t[:],
                                start=False,
                                stop=True,
                            )
                        else:
                            nc.tensor.matmul(
                                ps[:, jj, :],
                                waug_t[:, t, :],
                                caug_t[:],
                                start=True,
                                stop=True,
                            )
                    if "epi" in ablate:
                        continue
                    # qu = 1/(ps/512 + 1) = A/(1+d2); immediates only, 2 banks/op
                    qu = qu_pool.tile([P, 2, KC], F16)
                    _act_unsafe(
                        nc,
                        qu[:],
                        ps[:],
                        mybir.ActivationFunctionType.Reciprocal,
                        bias=1.0,
                        scale=1.0 / 512.0,
                    )
                    for jj in range(2):
                        j = 2 * jp + jj
                        # s = sum_k qu (tensor_scalar copy+accum, 4x mode)
                        s = stat_pool.tile([P, 1], F32)
                        z = z_pool.tile([P, KC], F16)
                        nc.vector.tensor_scalar(
                            z[:],
                            qu[:, jj, :],
                            1.0,
                            0.0,
                            MUL,
                            mybir.AluOpType.add,
                            accum_out=s[:],
                        )
                        rs = stat_pool.tile([P, 1], F32)
                        nc.vector.reciprocal_approx_fast(rs[:], s[:])
                        nc.vector.tensor_scalar_mul(ob[:, j, :], qu[:, jj, :], rs[:])
                # out-DMA from the Pool queue: keeps the SP stream (in-DMAs)
                # free of head-of-line blocking on the epilogue
                nc.gpsimd.dma_start(q[b], ob[:].rearrange("p j k -> p (j k)"))

    nc.compile()
    return nc


_NC_CACHE = {}


def _get_nc(**kw):
    key = tuple(sorted(kw.items()))
    if key not in _NC_CACHE:
        _NC_CACHE[key] = build_nc(**kw)
    return _NC_CACHE[key]


def prep_inputs(x, centroids, mm="fp8"):
    """Host-side layout prep + per-core sharding."""
    MMD = {"fp8": E4M3, "bf16": ml_dtypes.bfloat16}[mm]
    xf = np.ascontiguousarray(np.asarray(x, dtype=np.float32))
    cf = np.asarray(centroids, dtype=np.float32)
    n = xf.shape[0]
    ns = n // NCORES
    nblk = ns // BLK

    w512 = 512.0 / (1.0 + (xf * xf).sum(1))  # [N] f32
    xwv = (xf * w512[:, None]).astype(MMD)  # [N, D]
    cqT = np.ascontiguousarray((-2.0 * cf.T).astype(MMD))  # [D, K]

    wc = w512.astype(ml_dtypes.bfloat16)
    wr = (w512 - wc.astype(np.float32)).astype(ml_dtypes.bfloat16)
    waug = np.ascontiguousarray(np.stack([wc, wc, wr, wr], axis=0))  # [4, N] bf16

    csq = (cf * cf).sum(1)  # [K] f32 exact
    cc = csq.astype(ml_dtypes.bfloat16)
    cr = (csq - cc.astype(np.float32)).astype(ml_dtypes.bfloat16)
    caug = np.ascontiguousarray(np.stack([cc, cr, cc, cr], axis=0))  # [4, K] bf16

    in_maps = []
    for c in range(NCORES):
        # blocked xw: xw_b[b, p, c*BLK+m] = xw[row b*BLK+m, d=c*128+p]
        xs = xwv[c * ns : (c + 1) * ns]  # [ns, D]
        xb = np.ascontiguousarray(
            xs.reshape(nblk, BLK, NCH, P).transpose(0, 3, 2, 1).reshape(nblk, P, NCH * BLK)
        )
        in_maps.append(
            {
                "xw": xb,
                "cq": cqT,
                "waug": np.ascontiguousarray(waug[:, c * ns : (c + 1) * ns]),
                "caug": caug,
            }
        )
    return in_maps


def unblock_q(qb, n_rows=N_SHARD):
    """[nblk, P, G*KC] fp16 -> [n_rows, KC]"""
    nblk = n_rows // BLK
    return qb.reshape(nblk, P, G, KC).transpose(0, 2, 1, 3).reshape(n_rows, KC)


def kernel(x, centroids):
    nc = _get_nc()
    in_maps = prep_inputs(x, centroids)
    res = run_bass_kernel_spmd(nc, in_maps, core_ids=list(range(NCORES)))
    out = np.concatenate(
        [unblock_q(res.results[c]["q"]) for c in range(NCORES)], axis=0
    )
    return out.astype(np.float32)


if __name__ == "__main__":
    # smoke test with random data (no reference available standalone)
    rng = np.random.default_rng(0)
    x = rng.standard_normal((N_FULL, D), dtype=np.float32)
    c = rng.standard_normal((KC, D), dtype=np.float32)
    q = kernel(x, c)
    print("q", q.shape, q.dtype, q.sum(axis=1)[:4])
    xs = (x * x).sum(1)[:, None]
    cs = (c * c).sum(1)[None, :]
    d2 = xs + cs - 2.0 * (x @ c.T)
    qe = 1.0 / (1.0 + d2)
    qe /= qe.sum(1, keepdims=True)
    re = np.abs(q - qe) / np.maximum(np.abs(qe), 1e-30)
    print("rel err vs local reference:", re.max())
